# revision 33
# baseline (speedup 1.0000x reference)
"""GPT forward pass on 8 Trainium2 NeuronCores.

Sharding: cores 0-3 handle batch 0, cores 4-7 batch 1; within each 4-core
group the 1024 tokens are sequence-sharded 256/core. Activations are kept
feature-major (transposed) on chip so no on-device transposes are needed.
Per layer each core computes Q/K/V for its own tokens; K then V are
AllGathered (bf16, two pipelined collectives) within the 4-core group
(layer 0 computes K/V from the full h0 directly, no collective). The final
vocab projection is token-sharded: each core projects its own 256 tokens
against the full 32000-dim vocab with h-stationary matmuls (no final
AllGather); the output bias is added on the host.
"""

import os
import sys

for _p in ("/opt/trn_rl_repo", "/root/.axon_site/_ro/trn_rl_repo"):
    if os.path.isdir(_p) and _p not in sys.path:
        sys.path.insert(0, _p)

import ml_dtypes
import numpy as np

import concourse.bass as bass
import concourse.mybir as mybir
import concourse.tile as tile
from concourse import bacc
from concourse.bass_utils import run_bass_kernel_spmd

BF16 = ml_dtypes.bfloat16
f32 = mybir.dt.float32
bf16 = mybir.dt.bfloat16
AF = mybir.ActivationFunctionType
ALU = mybir.AluOpType

V, S, E, H, D, L = 32000, 1024, 512, 8, 64, 4
FF = 4 * E
B = 2
NC = 8
G = 4            # cores per batch group
TO = S // G      # tokens owned per core (256)
EPS = 1e-5
NKT = S // 128   # key tiles (8)
NFT = E // 128   # feature tiles (4)
NTT = TO // 128  # own-token tiles (2)
NMT = FF // 128  # FFN hidden tiles (16)
NV = 500         # vocab columns per projection chunk
NVC = V // NV    # 64 projection chunks

_cache = {}
COLLFREE = False

def build_nc(trace=False, rep=1):
    nc = bacc.Bacc("TRN2", target_bir_lowering=False, debug=False,
                   num_devices=1 if COLLFREE else NC)

    def din(name, shape, dt):
        return nc.dram_tensor(name, shape, dt, kind="ExternalInput").ap()

    io = dict(
        h0t_full=din("h0t_full", [E, S], bf16),
        h0t_own=din("h0t_own", [E, TO], f32),
        maskF=din("maskF", [S, TO], bf16),
        maskB=din("maskB", [128, NKT], f32),
        maskOwn=din("maskOwn", [2 * 128, TO], bf16),
        wq=din("wq", [L, E, H * D], bf16),
        wk=din("wk", [L, E, H * D], bf16),
        wv=din("wv", [L, E, H * D], bf16),
        wo=din("wo", [L, H * D, E], bf16),
        w1=din("w1", [L, E, FF], bf16),
        w2=din("w2", [L, FF, E], bf16),
        bq=din("bq", [L, E], f32),
        bk=din("bk", [L, E], f32),
        bo=din("bo", [L, E], f32),
        b1=din("b1", [L, FF], f32),
        b2=din("b2", [L, E], f32),
        g1=din("g1", [L, E], f32),
        be1=din("be1", [L, E], f32),
        g2=din("g2", [L, E], f32),
        be2=din("be2", [L, E], f32),
        wout=din("wout", [NVC, 128, NFT * NV], bf16),
        out=nc.dram_tensor("out", [NTT, NVC, 128, NV], bf16,
                           kind="ExternalOutput").ap(),
    )

    with tile.TileContext(nc) as tc:
        _body(nc, tc, io, rep=rep)
    nc.compile()
    return nc


class P:
    """pool handles"""


def _body(nc, tc, io, rep=1):
    from contextlib import ExitStack

    ctx = ExitStack()
    with ctx:
        p = P()
        p.w512 = ctx.enter_context(tc.tile_pool(name="w512", bufs=5))
        p.wff = ctx.enter_context(tc.tile_pool(name="wff", bufs=3))
        p.wout = ctx.enter_context(tc.tile_pool(name="pwout", bufs=6))
        p.kv = ctx.enter_context(tc.tile_pool(name="kv", bufs=1))
        p.act = ctx.enter_context(tc.tile_pool(name="act", bufs=1))
        p.a1 = ctx.enter_context(tc.tile_pool(name="a1p", bufs=NMT + 2))
        p.exp = ctx.enter_context(tc.tile_pool(name="exp", bufs=3))
        p.const = ctx.enter_context(tc.tile_pool(name="const", bufs=1))
        p.stat = ctx.enter_context(tc.tile_pool(name="stat", bufs=2))
        p.out = ctx.enter_context(tc.tile_pool(name="pout", bufs=3))
        p.mm = ctx.enter_context(tc.tile_pool(name="pmm", bufs=3, space="PSUM"))
        p.ua = ctx.enter_context(tc.tile_pool(name="uaug", bufs=4, space="PSUM"))
        p.misc = ctx.enter_context(tc.tile_pool(name="psmisc", bufs=1, space="PSUM"))
        p.dram = ctx.enter_context(tc.tile_pool(name="dram", bufs=2, space="DRAM"))

        def mmtile():
            return p.mm.tile([128, 512], f32, tag="mm", name="mm")

        # ---- constants ----
        ones_f = p.const.tile([128, 1], f32, tag="ones_f")
        nc.vector.memset(ones_f[:], 1.0)
        ones_r = p.const.tile([1, 128], f32, tag="ones_r")
        nc.vector.memset(ones_r[:], 1.0)
        zeros_r = p.const.tile([1, 512], bf16, tag="zeros_r")
        nc.vector.memset(zeros_r[:], 0.0)

        def ldvec(ap, name, n=NFT):
            t = p.const.tile([128, L * n], f32, tag=name)
            nc.sync.dma_start(t[:], ap.rearrange("l (k p) -> p (l k)", p=128)[:])
            return t

        bq_t = ldvec(io["bq"], "bq")
        bk_t = ldvec(io["bk"], "bk")
        bo_t = ldvec(io["bo"], "bo")
        b2_t = ldvec(io["b2"], "b2")
        g1_t = ldvec(io["g1"], "g1")
        be1_t = ldvec(io["be1"], "be1")
        g2_t = ldvec(io["g2"], "g2")
        be2_t = ldvec(io["be2"], "be2")
        b1_t = ldvec(io["b1"], "b1", n=NMT)

        mask_t = []
        for kt in range(NKT):
            m = p.const.tile([128, TO], bf16, tag=f"mask{kt}")
            nc.sync.dma_start(m[:], io["maskF"][kt * 128:(kt + 1) * 128, :])
            mask_t.append(m)
        maskb_t = p.const.tile([128, NKT], f32, tag="maskb")
        nc.sync.dma_start(maskb_t[:], io["maskB"][:, :])
        maskown_t = []
        for lt in range(NTT):
            mo = p.const.tile([128, TO], bf16, tag=f"masko{lt}")
            nc.sync.dma_start(mo[:], io["maskOwn"][lt * 128:(lt + 1) * 128, :])
            maskown_t.append(mo)

        # ---- initial hidden state ----
        ht = []
        for kf in range(NFT):
            t = p.act.tile([128, TO], f32, tag=f"ht{kf}")
            nc.sync.dma_start(t[:], io["h0t_own"][kf * 128:(kf + 1) * 128, :])
            ht.append(t)
        h0f_b = []
        for kf in range(NFT):
            t = p.const.tile([128, S], bf16, tag=f"h0fb{kf}")
            nc.sync.dma_start(t[:], io["h0t_full"][kf * 128:(kf + 1) * 128, :])
            h0f_b.append(t)

        def load_w512(ap, l):
            t = p.w512.tile([128, NFT * 512], bf16, tag="w512")
            nc.sync.dma_start(t[:].rearrange("p (k n) -> p k n", k=NFT),
                              ap[l].rearrange("(k p) n -> p k n", p=128)[:])
            return t

        htb = None
        for _rep in range(rep):
            _compute(nc, tc, io, p, locals())


def _compute(nc, tc, io, p, env):
    mmtile = env["mmtile"]
    ones_f = env["ones_f"]; ones_r = env["ones_r"]
    zeros_r = env["zeros_r"]
    bq_t = env["bq_t"]; bk_t = env["bk_t"]; bo_t = env["bo_t"]; b2_t = env["b2_t"]
    g1_t = env["g1_t"]; be1_t = env["be1_t"]; g2_t = env["g2_t"]; be2_t = env["be2_t"]
    b1_t = env["b1_t"]; mask_t = env["mask_t"]
    maskb_t = env["maskb_t"]; maskown_t = env["maskown_t"]
    ht = env["ht"]; h0f_b = env["h0f_b"]; load_w512 = env["load_w512"]
    if True:
        # ================= transformer layers =================
        for l in range(L):
            wq_t = load_w512(io["wq"], l)
            wk_t = load_w512(io["wk"], l)
            wv_t = load_w512(io["wv"], l)

            hb = []
            for kf in range(NFT):
                b = p.act.tile([128, TO], bf16, tag=f"hb{kf}")
                nc.vector.tensor_copy(b[:], ht[kf][:])
                hb.append(b)

            kt_all = []   # 4 tiles [128, S] bf16: gathered K^T
            vt_all = []   # 8 tiles [128, H*65] bf16: V with ones column per head
            if l == 0:
                for mf in range(NFT):
                    kt_t = p.kv.tile([128, S], bf16, tag=f"kt{mf}")
                    for c2 in range(S // 512):
                        ps = mmtile()
                        for kf in range(NFT):
                            nc.tensor.matmul(
                                ps[:],
                                wk_t[:, kf * 512 + mf * 128: kf * 512 + (mf + 1) * 128],
                                h0f_b[kf][:, c2 * 512:(c2 + 1) * 512],
                                start=(kf == 0), stop=(kf == NFT - 1))
                        nc.vector.tensor_scalar_add(
                            kt_t[:, c2 * 512:(c2 + 1) * 512], ps[:],
                            bk_t[:, l * NFT + mf: l * NFT + mf + 1])
                    kt_all.append(kt_t)
                for tt in range(NKT):
                    vt_t = p.kv.tile([128, H * 65], bf16, tag=f"vt{tt}")
                    ps = mmtile()
                    for kf in range(NFT):
                        nc.tensor.matmul(
                            ps[:],
                            h0f_b[kf][:, tt * 128:(tt + 1) * 128],
                            wv_t[:, kf * 512:(kf + 1) * 512],
                            start=(kf == 0), stop=(kf == NFT - 1))
                    nc.vector.tensor_copy(
                        vt_t.rearrange("p (h e) -> p h e", h=H)[:, :, 0:64],
                        ps.rearrange("p (h e) -> p h e", h=H)[:, :, :])
                    nc.vector.memset(
                        vt_t.rearrange("p (h e) -> p h e", h=H)[:, :, 64:65], 1.0)
                    vt_all.append(vt_t)
            klocal = []
            vtloc = []
            if l > 0:
                # K contribution first -> AllGather K while V projects
                contribK = p.dram.tile([E * TO], bf16, tag="contribK")
                for mf in range(NFT):
                    ps = mmtile()
                    for kf in range(NFT):
                        nc.tensor.matmul(
                            ps[:, :TO],
                            wk_t[:, kf * 512 + mf * 128: kf * 512 + (mf + 1) * 128],
                            hb[kf][:],
                            start=(kf == 0), stop=(kf == NFT - 1))
                    kl = p.act.tile([128, TO], bf16, tag=f"klocal{mf}", bufs=2)
                    nc.vector.tensor_scalar_add(
                        kl[:], ps[:, :TO], bk_t[:, l * NFT + mf: l * NFT + mf + 1])
                    klocal.append(kl)
                    nc.sync.dma_start(
                        contribK.rearrange("(p n) -> p n", p=E)[
                            mf * 128:(mf + 1) * 128, :],
                        kl[:])
                gathK = p.dram.tile([G, E * TO], bf16, tag="gathK")
                if COLLFREE:
                    for rr in range(G):
                        nc.sync.dma_start(gathK[rr], contribK[:])
                else:
                    nc.gpsimd.collective_compute(
                        "AllGather", ALU.bypass,
                        replica_groups=[[0, 1, 2, 3], [4, 5, 6, 7]],
                        ins=[contribK[:]], outs=[gathK[:]])

                contribV = p.dram.tile([TO * E], bf16, tag="contribV")
                for tt in range(NTT):
                    ps = mmtile()
                    for kf in range(NFT):
                        nc.tensor.matmul(
                            ps[:],
                            hb[kf][:, tt * 128:(tt + 1) * 128],
                            wv_t[:, kf * 512:(kf + 1) * 512],
                            start=(kf == 0), stop=(kf == NFT - 1))
                    vl = p.act.tile([128, 512], bf16, tag=f"vlocal{tt}", bufs=2)
                    nc.vector.tensor_copy(vl[:], ps[:])
                    nc.sync.dma_start(
                        contribV.rearrange("(t e) -> t e", e=E)[
                            tt * 128:(tt + 1) * 128, :],
                        vl[:])
                    vt_l = p.act.tile([128, H * 65], bf16, tag=f"vtloc{tt}",
                                      bufs=2)
                    nc.vector.tensor_copy(
                        vt_l.rearrange("p (h e) -> p h e", h=H)[:, :, 0:64],
                        vl[:].rearrange("p (h e) -> p h e", h=H)[:, :, :])
                    nc.vector.memset(
                        vt_l.rearrange("p (h e) -> p h e", h=H)[:, :, 64:65],
                        1.0)
                    vtloc.append(vt_l)
                gathV = p.dram.tile([G, TO * E], bf16, tag="gathV")
                if COLLFREE:
                    for rr in range(G):
                        nc.sync.dma_start(gathV[rr], contribV[:])
                else:
                    nc.gpsimd.collective_compute(
                        "AllGather", ALU.bypass,
                        replica_groups=[[0, 1, 2, 3], [4, 5, 6, 7]],
                        ins=[contribV[:]], outs=[gathV[:]])

                kg = gathK.rearrange("r (p c) -> p r c", p=E)
                vg = gathV.rearrange("r (t e) -> r t e", e=E)
                for mf in range(NFT):
                    kt_t = p.kv.tile([128, S], bf16, tag=f"kt{mf}")
                    nc.sync.dma_start(
                        kt_t[:].rearrange("p (r c) -> p r c", r=G),
                        kg[mf * 128:(mf + 1) * 128, :, :])
                    kt_all.append(kt_t)
                for tt in range(NKT):
                    vt_t = p.kv.tile([128, H * 65], bf16, tag=f"vt{tt}")
                    nc.sync.dma_start(
                        vt_t.rearrange("p (h e) -> p h e", h=H)[:, :, 0:64],
                        vg[tt // 2, (tt % 2) * 128:(tt % 2) * 128 + 128, :].rearrange(
                            "t (h e) -> t h e", h=H)[:])
                    nc.vector.memset(
                        vt_t.rearrange("p (h e) -> p h e", h=H)[:, :, 64:65], 1.0)
                    vt_all.append(vt_t)

            # Q^T [E, TO] bf16 (1/sqrt(D) folded into wq/bq on host)
            qt = []
            for mf in range(NFT):
                ps = mmtile()
                for kf in range(NFT):
                    nc.tensor.matmul(
                        ps[:, :TO],
                        wq_t[:, kf * 512 + mf * 128: kf * 512 + (mf + 1) * 128],
                        hb[kf][:],
                        start=(kf == 0), stop=(kf == NFT - 1))
                q = p.act.tile([128, TO], bf16, tag=f"qt{mf}")
                nc.vector.tensor_scalar_add(
                    q[:], ps[:, :TO], bq_t[:, l * NFT + mf: l * NFT + mf + 1])
                qt.append(q)

            wo_t = load_w512(io["wo"], l)
            w1a = p.wff.tile([128, 4096], bf16, tag="wff")
            nc.sync.dma_start(
                w1a[:].rearrange("p (k n) -> p k n", k=4),
                io["w1"][l][:, 0:1024].rearrange("(k p) n -> p k n", p=128)[:])
            w1b = p.wff.tile([128, 4096], bf16, tag="wff")
            nc.sync.dma_start(
                w1b[:].rearrange("p (k n) -> p k n", k=4),
                io["w1"][l][:, 1024:2048].rearrange("(k p) n -> p k n", p=128)[:])

            # ---- attention ----
            # scores for a head pair share one PSUM bank -> single exp per pair
            upair = [p.ua.tile([65, 512], f32, tag="uaug", name=f"ua{i}") for i in range(4)]
            for i in range(4):
                # open the bank's accumulation group across both heads
                nc.tensor.matmul(upair[i][:, :], zeros_r[0:1, 0:65],
                                 zeros_r[0:1, :], start=True, stop=False)
            if l > 0:
                # local pass: own K/V blocks straight from SBUF, overlaps
                # the AllGathers (the gathered-path mask zeroes these rows)
                for lt in range(NTT):
                    esl = p.exp.tile([128, H * TO], bf16, tag="expL", bufs=2)
                    for h in range(H):
                        sp = mmtile()
                        nc.tensor.matmul(
                            sp[:, :TO],
                            klocal[h // 2][64 * (h % 2):64 * (h % 2) + 64,
                                           lt * 128:(lt + 1) * 128],
                            qt[h // 2][64 * (h % 2):64 * (h % 2) + 64, :],
                            start=True, stop=True)
                        nc.scalar.activation(
                            esl[:, h * TO:(h + 1) * TO], sp[:, :TO], AF.Exp)
                        nc.vector.tensor_mul(
                            esl[:, h * TO:(h + 1) * TO],
                            esl[:, h * TO:(h + 1) * TO], maskown_t[lt][:])
                    for h in range(H):
                        nc.tensor.matmul(
                            upair[h // 2][:, 256 * (h % 2):256 * (h % 2) + 256],
                            vtloc[lt][:, h * 65:(h + 1) * 65],
                            esl[:, h * TO:(h + 1) * TO],
                            start=False, stop=False,
                            skip_group_check=True)
            for kt in range(NKT):
                es = p.exp.tile([128, H * TO], bf16, tag="expS")
                for h in range(H):
                    if h == 3 or h == 7:
                        sp = p.misc.tile([128, 512], f32, tag="psmisc",
                                         name=f"spm{l}_{kt}_{h}")
                    else:
                        sp = mmtile()
                    nc.tensor.matmul(
                        sp[:, :TO],
                        kt_all[h // 2][64 * (h % 2):64 * (h % 2) + 64,
                                       kt * 128:(kt + 1) * 128],
                        qt[h // 2][64 * (h % 2):64 * (h % 2) + 64, :],
                        start=True, stop=True)
                    if l == 0:
                        # per-element causal mask (diagonal blocks included)
                        nc.scalar.activation(
                            es[:, h * TO:(h + 1) * TO], sp[:, :TO], AF.Exp)
                        nc.vector.tensor_mul(
                            es[:, h * TO:(h + 1) * TO],
                            es[:, h * TO:(h + 1) * TO], mask_t[kt][:])
                    else:
                        # blocks are all-visible or all-masked per core:
                        # fold the mask into the exp as a -30 bias
                        nc.scalar.activation(
                            es[:, h * TO:(h + 1) * TO], sp[:, :TO], AF.Exp,
                            bias=maskb_t[:, kt:kt + 1])
                for h in range(H):
                    nc.tensor.matmul(
                        upair[h // 2][:, 256 * (h % 2):256 * (h % 2) + 256],
                        vt_all[kt][:, h * 65:(h + 1) * 65],
                        es[:, h * TO:(h + 1) * TO],
                        start=False,
                        stop=(kt == NKT - 1 and h % 2 == 1),
                        skip_group_check=True)

            # normalize heads -> conc^T [E, TO] bf16
            conc = []
            for mf in range(NFT):
                conc.append(p.act.tile([128, TO], bf16, tag=f"conc{mf}", name=f"conc{mf}"))
            for h in range(H):
                rec = p.stat.tile([1, TO], f32, tag="rec")
                nc.vector.reciprocal(
                    rec[:], upair[h // 2][64:65, 256 * (h % 2):256 * (h % 2) + 256])
                rb = p.misc.tile([64, TO], f32, tag="psmisc")
                nc.tensor.matmul(rb[:], ones_r[0:1, 0:64], rec[:],
                                 start=True, stop=True)
                rbs = p.stat.tile([64, TO], f32, tag="rbs")
                nc.vector.tensor_copy(rbs[:], rb[:])
                nc.vector.tensor_mul(
                    conc[h // 2][64 * (h % 2):64 * (h % 2) + 64, :],
                    upair[h // 2][0:64, 256 * (h % 2):256 * (h % 2) + 256],
                    rbs[:])

            # ---- mha^T + residual + LN1 ----
            res1 = []
            for mf in range(NFT):
                ps = mmtile()
                for kf in range(NFT):
                    nc.tensor.matmul(
                        ps[:, :TO],
                        wo_t[:, kf * 512 + mf * 128: kf * 512 + (mf + 1) * 128],
                        conc[kf][:],
                        start=(kf == 0), stop=(kf == NFT - 1))
                r = p.act.tile([128, TO], f32, tag=f"res1{mf}")
                nc.vector.tensor_scalar_add(
                    r[:], ps[:, :TO], bo_t[:, l * NFT + mf: l * NFT + mf + 1])
                nc.vector.tensor_add(r[:], r[:], ht[mf][:])
                res1.append(r)

            ln1f, ln1b = _layernorm(nc, p, ones_f, ones_r, res1,
                                    g1_t, be1_t, l, "ln1", mmtile)

            # ---- FFN ----
            w2a = p.wff.tile([128, 4096], bf16, tag="wff")
            nc.sync.dma_start(
                w2a[:].rearrange("p (k n) -> p k n", k=8),
                io["w2"][l][0:1024, :].rearrange("(k p) n -> p k n", p=128)[:])
            w2b = p.wff.tile([128, 4096], bf16, tag="wff")
            nc.sync.dma_start(
                w2b[:].rearrange("p (k n) -> p k n", k=8),
                io["w2"][l][1024:2048, :].rearrange("(k p) n -> p k n", p=128)[:])

            a1 = []
            for mt in range(NMT):
                wsrc = w1a if mt < 8 else w1b
                moff = mt % 8
                ps = mmtile()
                for kf in range(NFT):
                    nc.tensor.matmul(
                        ps[:, :TO],
                        wsrc[:, kf * 1024 + moff * 128: kf * 1024 + (moff + 1) * 128],
                        ln1b[kf][:],
                        start=(kf == 0), stop=(kf == NFT - 1))
                a = p.a1.tile([128, TO], bf16, tag="a1")
                nc.scalar.activation(
                    a[:], ps[:, :TO], AF.Relu,
                    bias=b1_t[:, l * NMT + mt: l * NMT + mt + 1])
                a1.append(a)

            res2 = []
            for mf in range(NFT):
                ps = mmtile()
                for kt2 in range(NMT):
                    wsrc = w2a if kt2 < 8 else w2b
                    koff = kt2 % 8
                    nc.tensor.matmul(
                        ps[:, :TO],
                        wsrc[:, koff * 512 + mf * 128: koff * 512 + (mf + 1) * 128],
                        a1[kt2][:],
                        start=(kt2 == 0), stop=(kt2 == NMT - 1))
                r = p.act.tile([128, TO], f32, tag=f"res2{mf}")
                nc.vector.tensor_scalar_add(
                    r[:], ps[:, :TO], b2_t[:, l * NFT + mf: l * NFT + mf + 1])
                nc.vector.tensor_add(r[:], r[:], ln1f[mf][:])
                res2.append(r)

            ht, htb = _layernorm(nc, p, ones_f, ones_r, res2,
                                 g2_t, be2_t, l, "ln2", mmtile)

        # ================= token-sharded vocab projection =================
        # out[t, v] = sum_e h[e, t] * wout[e, v] for the core's own 256 tokens.
        # h blocks are the stationary operand (reused across 4 chunk matmuls
        # per LDWEIGHTS); wout streams from HBM chunk by chunk.
        def wchunk(c):
            wt = p.wout.tile([128, NFT * NV], bf16, tag="wout")
            nc.sync.dma_start(wt[:], io["wout"][c])
            return wt

        r = 0
        for cg in range(NVC // 4):
            cs = [cg * 4 + i for i in range(4)]
            wtiles = [wchunk(c) for c in cs]
            for tt in range(NTT):
                # 4 chunks share one stationary h block per kf pass, so the
                # PE issues 4 streaming matmuls per LDWEIGHTS; the two PSUM
                # bank quads (uaug / mm+misc) ping-pong across rounds.
                pss = []
                for i in range(4):
                    if r % 2 == 0:
                        pss.append(p.ua.tile([128, NV], f32, tag="uaug",
                                             name=f"pj{r}_{i}"))
                    elif i < 3:
                        pss.append(p.mm.tile([128, 512], f32, tag="mm",
                                             name=f"pj{r}_{i}"))
                    else:
                        pss.append(p.misc.tile([128, NV], f32, tag="psmisc",
                                               name=f"pj{r}_{i}"))
                for kf in range(NFT):
                    for i in range(4):
                        nc.tensor.matmul(
                            pss[i][:, :NV],
                            htb[kf][:, tt * 128:(tt + 1) * 128],
                            wtiles[i][:, kf * NV:(kf + 1) * NV],
                            start=(kf == 0), stop=(kf == NFT - 1),
                            skip_group_check=True)
                for i, c in enumerate(cs):
                    ot = p.out.tile([128, NV], bf16, tag="outsb")
                    if i % 2 == 0:
                        nc.vector.tensor_copy(ot[:], pss[i][:, :NV])
                    else:
                        nc.scalar.mul(ot[:], pss[i][:, :NV], 1.0)
                    nc.sync.dma_start(io["out"][tt, c], ot[:])
                r += 1


def _layernorm(nc, p, ones_f, ones_r, res, g_t, b_t, l, name, mmtile):
    """Feature-major layernorm over NFT [128, TO] fp32 tiles -> (f32, bf16)."""
    sums = p.misc.tile([33, TO], f32, tag="psmisc")
    for kf in range(NFT):
        nc.tensor.matmul(sums[0:1, :], ones_f[:, :], res[kf][:],
                         start=(kf == 0), stop=(kf == NFT - 1))
    for kf in range(NFT):
        sq = p.act.tile([128, TO], f32, tag="sq", bufs=2)
        nc.scalar.activation(sq[:], res[kf][:], AF.Square)
        nc.tensor.matmul(sums[32:33, :], ones_f[:, :], sq[:],
                         start=(kf == 0), stop=(kf == NFT - 1))
    sv = p.stat.tile([1, 6 * TO], f32, tag="stat")
    mu = sv[:, 0:TO]
    musq = sv[:, TO:2 * TO]
    var = sv[:, 2 * TO:3 * TO]
    std = sv[:, 3 * TO:4 * TO]
    rstd = sv[:, 4 * TO:5 * TO]
    murstd = sv[:, 5 * TO:6 * TO]
    nc.scalar.mul(mu, sums[0:1, :], 1.0 / E)
    nc.vector.tensor_mul(musq, mu, mu)
    nc.vector.tensor_scalar(var, sums[32:33, :], 1.0 / E, EPS,
                            ALU.mult, ALU.add)
    nc.vector.tensor_sub(var, var, musq)
    # rstd = exp(-0.5*ln(var)): keeps ACT on the natural_log_exp table set
    # (a scalar.sqrt here would force a table swap against attention's Exp)
    nc.scalar.activation(std, var, AF.Ln)
    nc.scalar.activation(rstd, std, AF.Exp, scale=-0.5)
    nc.vector.tensor_mul(murstd, mu, rstd)
    rb = mmtile()
    nc.tensor.matmul(rb[:, :TO], ones_r[:, :], rstd, start=True, stop=True)
    mb = mmtile()
    nc.tensor.matmul(mb[:, :TO], ones_r[:, :], murstd, start=True, stop=True)
    outf, outb = [], []
    for kf in range(NFT):
        t = p.act.tile([128, TO], f32, tag=f"{name}f{kf}", bufs=2)
        nc.vector.tensor_mul(t[:], res[kf][:], rb[:, :TO])
        nc.vector.tensor_sub(t[:], t[:], mb[:, :TO])
        nc.vector.tensor_scalar(
            t[:], t[:],
            g_t[:, l * NFT + kf: l * NFT + kf + 1],
            b_t[:, l * NFT + kf: l * NFT + kf + 1],
            ALU.mult, ALU.add)
        b = p.act.tile([128, TO], bf16, tag=f"{name}b{kf}", bufs=2)
        nc.vector.tensor_copy(b[:], t[:])
        outf.append(t)
        outb.append(b)
    return outf, outb


def _prep_inputs(x, tok_emb, pos_emb, Wq, bq, Wk, bk, Wv, bv, Wo, bo,
                 W1, b1, W2, b2, ln1_g, ln1_b, ln2_g, ln2_b, Wout, bout):
    """Host-side sharding: returns in_maps for the 8 cores."""
    x = np.asarray(x)
    h0 = np.asarray(tok_emb)[x] + np.asarray(pos_emb)[None, :, :]   # [B,S,E] f32
    h0t = np.ascontiguousarray(np.transpose(h0, (0, 2, 1)))          # [B,E,S]

    scale = 1.0 / np.sqrt(D)
    wq_h = (np.transpose(np.asarray(Wq), (0, 2, 1, 3)).reshape(L, E, H * D)
            * scale).astype(BF16)
    wk_h = np.transpose(np.asarray(Wk), (0, 2, 1, 3)).reshape(L, E, H * D).astype(BF16)
    wv_h = np.transpose(np.asarray(Wv), (0, 2, 1, 3)).reshape(L, E, H * D).astype(BF16)
    wo_h = np.asarray(Wo).astype(BF16)
    w1_h = np.asarray(W1).astype(BF16)
    w2_h = np.asarray(W2).astype(BF16)
    bq_h = (np.asarray(bq).reshape(L, H * D) * scale).astype(np.float32)
    bk_h = np.asarray(bk).reshape(L, H * D).astype(np.float32)
    bv_c = np.asarray(bv).reshape(L, H * D).astype(np.float32)
    bo_eff = (np.asarray(bo) + np.einsum("lc,lce->le", bv_c, np.asarray(Wo))
              ).astype(np.float32)
    # pack wout into per-chunk on-chip tile layout: [c][p][k*NV+n] =
    # Wout[k*128+p, c*NV+n] -> contiguous 4KB DMA lines
    wout_np = np.ascontiguousarray(
        np.asarray(Wout).astype(BF16)
        .reshape(NFT, 128, NVC, NV)
        .transpose(2, 1, 0, 3)
        .reshape(NVC, 128, NFT * NV))
    common = dict(
        wq=wq_h, wk=wk_h, wv=wv_h, wo=wo_h, w1=w1_h, w2=w2_h,
        bq=bq_h, bk=bk_h, bo=bo_eff,
        b1=np.asarray(b1).astype(np.float32),
        b2=np.asarray(b2).astype(np.float32),
        g1=np.asarray(ln1_g).astype(np.float32),
        be1=np.asarray(ln1_b).astype(np.float32),
        g2=np.asarray(ln2_g).astype(np.float32),
        be2=np.asarray(ln2_b).astype(np.float32),
        wout=wout_np,
    )

    key_pos = np.arange(S)[:, None]
    in_maps = []
    for c in range(NC):
        b, j = c // G, c % G
        qpos = j * TO + np.arange(TO)[None, :]
        mask = (key_pos <= qpos).astype(BF16)            # [S, TO]
        maskown = np.ascontiguousarray(mask[j * TO:(j + 1) * TO, :])
        # gathered-path visibility per 128-key block: fully visible (0.0)
        # only strictly below this core's own rows; own rows come from the
        # local pass, everything else exp(-30)-masked
        maskb = np.full((128, NKT), -30.0, np.float32)
        maskb[:, :2 * j] = 0.0
        in_maps.append(dict(
            common,
            h0t_full=h0t[b].astype(BF16),
            h0t_own=np.ascontiguousarray(
                h0t[b][:, j * TO:(j + 1) * TO]).astype(np.float32),
            maskF=mask,
            maskB=maskb,
            maskOwn=maskown,
        ))
    return in_maps


def _finish_output(res, bout):
    bout = np.asarray(bout, dtype=np.float32)
    logits = np.empty((B, S, V), dtype=np.float32)
    for c in range(NC):
        b, j = c // G, c % G
        o = np.asarray(res.results[c]["out"], dtype=np.float32)
        o = o.transpose(0, 2, 1, 3).reshape(TO, V)   # [tt,c,p,n] -> [t, v]
        logits[b, j * TO:(j + 1) * TO, :] = o + bout[None, :]
    return logits


def kernel(**inputs):
    if "nc" not in _cache:
        _cache["nc"] = build_nc()
    nc = _cache["nc"]
    inputs = {k: np.asarray(v) for k, v in inputs.items()}
    in_maps = _prep_inputs(**inputs)
    res = run_bass_kernel_spmd(nc, in_maps, list(range(NC)))
    return _finish_output(res, inputs["bout"])
